# revision 1
# baseline (speedup 1.0000x reference)
"""Multi-head attention (B=2, S=2048, D=1024, H=16, dk=dv=64) on 8 trn2 cores.

Sharding: (batch, head-quad) -> core.  Core i handles batch i//4 and the 4
heads [4*(i%4), 4*(i%4)+4).  Each core computes its partial output
context_h @ W_O[h-slice] summed over its 4 heads; the host sums the 4
partials per batch (the "all-reduce" of the row-sharded output projection).

Per-core kernel (all matmuls bf16 in / fp32 accumulate):
  1. Q^T, K^T projections into [d, s] layout (lhsT = W chunks, rhs = x^T),
     V projection into [s, d] layout (lhsT = x^T chunks, rhs = W_V), with an
     extra ones-column appended per head so the attention A@V matmul also
     produces the softmax denominators for free.
  2. Per (head, 512-query-block): scores^T tiles [sk=128, sq=512] via
     lhsT=K^T tile, rhs=Q^T block (causal: only lower tiles), exp on ACT
     (scale=1/8 folded in, no max-subtraction needed: |logits| <= ~6),
     multiplicative 0/1 causal mask on the diagonal tile groups,
     context^T accumulation via lhsT=V[sk,65], rhs=P^T.
  3. Normalize: reciprocal of the sums row, broadcast across partitions
     (gpsimd partition_broadcast), multiply -> ctx^T bf16.
  4. Output projection: lhsT = ctx^T chunks [128, s-tile], rhs = W_O chunks.

Pipelining: the two heads of a pair are interleaved and A@V consumers run 3
tasks behind the scores/exp producers (SBUF-buffered P^T tiles), so the
in-order PE queue never waits on ACT's exp latency; scores have 3 PSUM
slots of lookahead; output-projection matmuls are woven in at pair
boundaries, borrowing the freed ctx PSUM slots.
"""

import os
import numpy as np
import ml_dtypes

import concourse.bacc as bacc
import concourse.tile as tile
import concourse.mybir as mybir
import concourse.bass_utils as bass_utils
from concourse.bass import ds

B, S, D, H, DK = 2, 2048, 1024, 16, 64
N_CORES = 8
HPC = 4            # heads per core
NCH = 8            # d-model chunks of 128
NB = 4             # query blocks of 512
BLK = 512
NT = 16            # s tiles of 128
VW = DK + 1        # V columns per head incl. ones column

DT = mybir.dt.bfloat16
NP_DT = ml_dtypes.bfloat16
F32 = mybir.dt.float32

TRACE = False      # set True (or BASS_TRACE=1) to capture an NTFF profile
LAST_RESULTS = None

_CACHED_NC = None


def _build_program():
    nc = bacc.Bacc("TRN2", target_bir_lowering=False, debug=False,
                   enable_asserts=False, num_devices=N_CORES)

    xq_d = nc.dram_tensor("xq_t", [NCH, 128, S], DT, kind="ExternalInput")
    xk_d = nc.dram_tensor("xk_t", [NCH, 128, S], DT, kind="ExternalInput")
    xv_d = nc.dram_tensor("xv_t", [NCH, 128, S], DT, kind="ExternalInput")
    wq_d = nc.dram_tensor("wq", [128, NCH, HPC * DK], DT, kind="ExternalInput")
    wk_d = nc.dram_tensor("wk", [128, NCH, HPC * DK], DT, kind="ExternalInput")
    wv_d = nc.dram_tensor("wv", [128, NCH, HPC * DK], DT, kind="ExternalInput")
    wo_d = nc.dram_tensor("wo", [128, 2, D], DT, kind="ExternalInput")
    mask_d = nc.dram_tensor("mask01", [128, 4, 1024], DT, kind="ExternalInput")
    out_d = nc.dram_tensor("out_partial", [S, D], F32, kind="ExternalOutput")

    with tile.TileContext(nc) as tc:
        _body(tc, xq_d, xk_d, xv_d, wq_d, wk_d, wv_d, wo_d, mask_d, out_d)
    nc.compile()
    return nc


def _body(tc, xq_d, xk_d, xv_d, wq_d, wk_d, wv_d, wo_d, mask_d, out_d):
    nc = tc.nc

    with (
        tc.tile_pool(name="consts", bufs=1) as consts,
        tc.tile_pool(name="persist", bufs=1) as persist,
        tc.tile_pool(name="small", bufs=3) as small,
    ):
        # ---- constants ----
        wq_sb = consts.tile([128, NCH, HPC * DK], DT)
        wk_sb = consts.tile([128, NCH, HPC * DK], DT)
        wv_sb = consts.tile([128, NCH, HPC * DK], DT)
        wo_sb = consts.tile([128, 2, D], DT)
        mask_sb = consts.tile([128, 4, 1024], DT)

        # ---- persistent activations ----
        qt_sb = persist.tile([128, 2, S], DT)        # Q^T, pair-major
        kt_sb = persist.tile([128, 2, S], DT)        # K^T
        v_sb = persist.tile([128, NT, HPC * VW], DT)  # V + ones cols
        ctxt_sb = persist.tile([128, 2, S], DT)      # context^T

        for hh in range(HPC):
            nc.vector.memset(v_sb[:, :, hh * VW + DK: hh * VW + DK + 1], 1.0)

        # ---- Q^T/K^T projections (DMA order: wq, xq, mask, wk, xk) ----
        # Block-0 (and block-1 pair-0) scores/exp tasks are woven in between
        # the KT projection blocks so the ACT exp stream starts as soon as
        # the Q/K DMAs allow.
        with (
            tc.tile_pool(name="pt", bufs=20) as pt_pool,
            tc.tile_pool(name="osb", bufs=3) as out_pool,
            tc.tile_pool(name="psum_sc", bufs=3, space="PSUM") as sc_pool,
            tc.tile_pool(name="psum_ctx", bufs=2, space="PSUM") as ctx_pool,
        ):
            st = dict(sc_pool=sc_pool, ctx_pool=ctx_pool,
                      pt_pool=pt_pool, out_pool=out_pool, small=small,
                      qt=qt_sb, kt=kt_sb, v=v_sb, ctxt=ctxt_sb,
                      mask=mask_sb, wo=wo_sb, out_d=out_d, nc=nc,
                      EXP=mybir.ActivationFunctionType.Exp,
                      MUL=mybir.AluOpType.mult)

            queue = []      # produced-but-unconsumed (blk, hp, skt, pt)

            def produce_step(blk, hp, skt):
                pt = _produce(st, blk, hp, skt)
                queue.append((blk, hp, skt, pt))

            with (
                tc.tile_pool(name="xq", bufs=1) as xq_pool,
                tc.tile_pool(name="xk", bufs=1) as xk_pool,
            ):
                xq_sb = xq_pool.tile([128, NCH, S], DT)
                xk_sb = xk_pool.tile([128, NCH, S], DT)
                nc.sync.dma_start(wq_sb[:], wq_d[:])
                for c in range(NCH):
                    nc.sync.dma_start(xq_sb[:, c, :], xq_d[c])
                nc.sync.dma_start(mask_sb[:], mask_d[:])
                nc.sync.dma_start(wk_sb[:], wk_d[:])
                for c in range(NCH):
                    nc.sync.dma_start(xk_sb[:, c, :], xk_d[c])
                nc.sync.dma_start(wv_sb[:], wv_d[:])

                def proj_block(dst, w_sb, x_sb, blk):
                    for p in range(2):
                        ps = ctx_pool.tile([128, BLK], F32, name="qkps", tag="ctx")
                        for c in range(NCH):
                            nc.tensor.matmul(
                                ps[:],
                                lhsT=w_sb[:, c, ds(128 * p, 128)],
                                rhs=x_sb[:, c, ds(BLK * blk, BLK)],
                                start=(c == 0), stop=(c == NCH - 1))
                        nc.vector.tensor_copy(dst[:, p, ds(BLK * blk, BLK)], ps[:])

                for blk in range(NB):
                    proj_block(qt_sb, wq_sb, xq_sb, blk)
                proj_block(kt_sb, wk_sb, xk_sb, 0)
                for skt in range(4):
                    produce_step(0, 0, skt)
                proj_block(kt_sb, wk_sb, xk_sb, 1)
                for skt in range(4):
                    produce_step(0, 1, skt)
                proj_block(kt_sb, wk_sb, xk_sb, 2)
                for skt in range(8):
                    produce_step(1, 0, skt)
                proj_block(kt_sb, wk_sb, xk_sb, 3)

            # ---- V projection ----
            # xv reuses xq's freed ring zone, so its DMAs only wait on the
            # QT matmuls (~25us); the V matmuls then run densely on the
            # still-unused ctx psum slots.
            with tc.tile_pool(name="xv", bufs=1) as xv_pool:
                xv_sb = xv_pool.tile([128, NCH, S], DT)
                for c in range(NCH):
                    nc.sync.dma_start(xv_sb[:, c, :], xv_d[c])
                nc.sync.dma_start(wo_sb[:], wo_d[:])
                for t in range(NT):
                    ps = ctx_pool.tile([128, HPC * DK], F32, name="vps", tag="ctx")
                    for c in range(NCH):
                        nc.tensor.matmul(
                            ps[:],
                            lhsT=xv_sb[:, c, ds(128 * t, 128)],
                            rhs=wv_sb[:, c, :],
                            start=(c == 0), stop=(c == NCH - 1))
                    dst = v_sb[:, t, :].rearrange(
                        "p (hh e) -> p hh e", hh=HPC)[:, :, 0:DK]
                    srcap = ps[:].rearrange("p (hh e) -> p hh e", hh=HPC)
                    nc.vector.tensor_copy(dst, srcap)

            # ---- attention stream ----
            ctx_maps = {}   # (blk, hp) -> {h: psum tile}

            def do_consume():
                blk, hp, skt, pt = queue.pop(0)
                ctxps = ctx_maps.setdefault((blk, hp), {})
                _consume(st, (blk, hp, skt, pt), ctxps)
                if skt == 4 * (blk + 1) - 1:  # pair complete
                    for hh2 in range(2):
                        _normalize(st, blk, hp, hh2, ctxps)
                    del ctx_maps[(blk, hp)]
                    if blk > 0:  # lagged output projection, 2 s-tiles
                        for t in range(4 * (blk - 1) + 2 * hp,
                                       4 * (blk - 1) + 2 * hp + 2):
                            _outproj_tile(st, t)

            rest = [(1, 1, skt) for skt in range(8)]
            rest += [(blk, hp, skt)
                     for blk in range(2, NB)
                     for hp in range(2)
                     for skt in range(4 * (blk + 1))]
            for blk, hp, skt in rest:
                produce_step(blk, hp, skt)
                drained = 0
                while len(queue) > 4 and drained < 2:
                    do_consume()
                    drained += 1
            while queue:
                do_consume()
            for t in range(4 * (NB - 1), 4 * NB):
                _outproj_tile(st, t)


def _produce(st, blk, hp, skt):
    """Scores matmuls + exp (+ causal mask) for one (pair, sk-tile) task.

    The two heads of the pair sit on disjoint PE row groups (lhsT base
    partitions 0 and 64), so their back-to-back scores matmuls execute
    concurrently in the array; both heads' P^T share one [128,1024] tile
    (head h2 in columns 512*h2..512*h2+512) and one exp ACTIVATE."""
    nc = st["nc"]
    sc = st["sc_pool"].tile([128, 1024], F32, name="sc", tag="sc")
    for h2 in range(2):
        nc.tensor.matmul(
            sc[:, ds(512 * h2, 512)],
            lhsT=st["kt"][ds(64 * h2, 64), hp, ds(128 * skt, 128)],
            rhs=st["qt"][ds(64 * h2, 64), hp, ds(BLK * blk, BLK)],
            start=True, stop=True)
    pt = st["pt_pool"].tile([128, 1024], DT, name="pt", tag="pt")
    nc.scalar.activation(pt[:], sc[:], st["EXP"], scale=0.125)
    if skt >= 4 * blk:  # diagonal tiles: zero the masked region (both heads)
        u = skt - 4 * blk
        nc.vector.tensor_tensor(
            pt[:], pt[:], st["mask"][:, u, :], st["MUL"])
    return pt


def _consume(st, task, ctxps):
    """A@V accumulation for one produced task (both heads of the pair)."""
    nc = st["nc"]
    blk, hp, skt, pt = task
    last = 4 * (blk + 1) - 1
    for h2 in range(2):
        h = 2 * hp + h2
        if h not in ctxps:
            ctxps[h] = st["ctx_pool"].tile(
                [128, BLK], F32, name=f"ctx{h2}", tag="ctx")
        nc.tensor.matmul(
            ctxps[h][0:DK + 1, :],
            lhsT=st["v"][:, skt, ds(h * VW, VW)],
            rhs=pt[:, ds(512 * h2, 512)],
            start=(skt == 0), stop=(skt == last))


def _normalize(st, blk, hp, h2, ctxps):
    """ctx rows 0..63 scaled by 1/row64 -> ctx^T bf16.

    The ctx PSUM slot is released by two quick DVE copies (sums row +
    ctx rows into SBUF); the reciprocal/broadcast/multiply then run off
    the critical path so the next pair's A@V is not stalled."""
    nc = st["nc"]
    h = 2 * hp + h2
    # custom-DVE ops read garbage from PSUM -> plain-copy the sums row to
    # SBUF first (DVE copy of [1,512] is cheap; DVE reads PSUM fine).
    sums = st["small"].tile([1, BLK], F32, name="sums", tag="sums")
    nc.vector.tensor_copy(sums[:], ctxps[h][ds(DK, 1), :])
    raw = st["small"].tile([64, BLK], F32, name="raw", tag="raw")
    nc.vector.tensor_copy(raw[:], ctxps[h][0:64, :])
    r = st["small"].tile([1, BLK], F32, name="r", tag="r")
    nc.vector.reciprocal_approx_fast(out=r[:], in_=sums[:])
    bc = st["small"].tile([64, BLK], F32, name="bc", tag="bc")
    nc.gpsimd.partition_broadcast(bc[:], r[:])
    nc.vector.tensor_tensor(
        st["ctxt"][ds(64 * h2, 64), hp, ds(BLK * blk, BLK)],
        raw[:], bc[:], st["MUL"])


def _outproj_tile(st, t):
    nc = st["nc"]
    ob = st["out_pool"].tile([128, D], F32, name="ob", tag="ob")
    for nb in range(2):
        pp = st["ctx_pool"].tile([128, 512], F32, name="pp", tag="ctx")
        for cc in range(2):
            nc.tensor.matmul(
                pp[:],
                lhsT=st["ctxt"][:, cc, ds(128 * t, 128)],
                rhs=st["wo"][:, cc, ds(512 * nb, 512)],
                start=(cc == 0), stop=(cc == 1))
        nc.vector.tensor_copy(ob[:, ds(512 * nb, 512)], pp[:])
    nc.sync.dma_start(st["out_d"][ds(128 * t, 128), :], ob[:])


def _make_mask():
    # mask01[i, m, 512*h2 + q] = 1.0 iff key (128*m + i) <= query q: pattern
    # for diagonal sk-tile offset m within the 512-block, duplicated in both
    # halves (the two heads of a pair share one P^T tile).
    i = np.arange(128)[:, None]
    q = np.arange(512)[None, :]
    pats = [np.concatenate([p, p], axis=1)
            for p in ((128 * m + i <= q) for m in range(4))]
    return np.stack(pats, axis=1).astype(NP_DT)


def _prep_core_inputs(inputs, core):
    b = core // 4
    h0 = HPC * (core % 4)
    c0, c1 = h0 * DK, (h0 + HPC) * DK
    f32 = np.float32

    def t_chunks(x):  # [S, D] -> [NCH, 128, S]
        xt = np.ascontiguousarray(np.asarray(x, f32).T)
        return xt.reshape(NCH, 128, S).astype(NP_DT)

    return {
        "xq_t": t_chunks(inputs["input_Q"][b]),
        "xk_t": t_chunks(inputs["input_K"][b]),
        "xv_t": t_chunks(inputs["input_V"][b]),
        "wq": np.ascontiguousarray(np.asarray(inputs["W_Q"], f32)[:, c0:c1].reshape(NCH, 128, HPC * DK).transpose(1, 0, 2)).astype(NP_DT),
        "wk": np.ascontiguousarray(np.asarray(inputs["W_K"], f32)[:, c0:c1].reshape(NCH, 128, HPC * DK).transpose(1, 0, 2)).astype(NP_DT),
        "wv": np.ascontiguousarray(np.asarray(inputs["W_V"], f32)[:, c0:c1].reshape(NCH, 128, HPC * DK).transpose(1, 0, 2)).astype(NP_DT),
        "wo": np.ascontiguousarray(np.asarray(inputs["W_O"], f32)[c0:c1, :].reshape(2, 128, D).transpose(1, 0, 2)).astype(NP_DT),
        "mask01": _make_mask(),
    }


def get_program():
    global _CACHED_NC
    if _CACHED_NC is None:
        _CACHED_NC = _build_program()
    return _CACHED_NC


def kernel(**inputs):
    global LAST_RESULTS
    nc = get_program()
    in_maps = [_prep_core_inputs(inputs, core) for core in range(N_CORES)]
    res = bass_utils.run_bass_kernel_spmd(
        nc, in_maps, core_ids=list(range(N_CORES)),
        trace=TRACE or bool(int(os.environ.get("BASS_TRACE", "0") or 0)))
    LAST_RESULTS = res
    out = np.zeros((B, S, D), np.float32)
    for core in range(N_CORES):
        out[core // 4] += res.results[core]["out_partial"]
    return out



# revision 11
# speedup vs baseline: 1.2708x; 1.2708x over previous
"""Multi-head attention (B=2, S=2048, D=1024, H=16, dk=dv=64) on 8 trn2 cores.

Sharding: (batch, head-quad) -> core.  Core i handles batch i//4 and the 4
heads [4*(i%4), 4*(i%4)+4).  Each core computes its partial output
context_h @ W_O[h-slice] summed over its 4 heads; the host sums the 4
partials per batch (the "all-reduce" of the row-sharded output projection).

v2 schedule (vs the 205us baseline): the kernel is a single interleaved
stream built around keeping ScalarE's exp pipeline (the serial softmax
resource, ~1.1us per 128x1024 tile) and the PE dense simultaneously:

  - DMA: need-ordered 1MB column-block descriptors (tri, wk, xk[q0], wq,
    xq[q0], wv, xv[q0], xq[q1], xk[q1], wo, xv[q1], xq[q2], xk[q2], ...)
    so K/Q proj block 0 and the first scores/exp start ~15us earlier.
  - PSUM: sc 2x2 banks (scores), ctx 2x1 (A@V accumulators, live per pair),
    fill 2x1 (QKV proj + out-proj groups) -- so "fill" matmuls weave into
    the PE queue mid-pair instead of only at pair boundaries.
  - Causal column restriction: diagonal tasks compute scores/exp/A@V only
    for q >= 128*u (the visible columns); the mask shrinks to one 128x128
    triangle multiply per head (DVE 2x mode) instead of 128x1024.
  - Fills (K/Q proj blocks 1-3, V proj tiles, lagged out-proj tiles) are
    generators stepped ~2x per task between produce/consume so the PE never
    idles long enough to re-throttle (HAM) and ACT never starves.
  - Normalize: one merged [65,512] PSUM->SBUF copy (sums row + ctx rows)
    releases the ctx bank fast; reciprocal/broadcast/multiply off-path.

All matmuls bf16 in / fp32 accumulate; scores pairs run concurrently on
disjoint 64-row PE groups; A@V uses the ones-column trick so the softmax
denominators fall out of the same matmul.
"""

import os
import numpy as np
import ml_dtypes

import concourse.bacc as bacc
import concourse.tile as tile
import concourse.mybir as mybir
import concourse.bass_utils as bass_utils
from concourse.bass import ds

B, S, D, H, DK = 2, 2048, 1024, 16, 64
N_CORES = 8
HPC = 4            # heads per core
NCH = 8            # d-model chunks of 128
NB = 4             # query blocks of 512
BLK = 512
NT = 16            # s tiles of 128
VW = DK + 1        # V columns per head incl. ones column

DT = mybir.dt.bfloat16
NP_DT = ml_dtypes.bfloat16
F32 = mybir.dt.float32

TRACE = False
LAST_RESULTS = None

_CACHED_NC = None


def _build_program():
    nc = bacc.Bacc("TRN2", target_bir_lowering=False, debug=False,
                   enable_asserts=False, num_devices=N_CORES)

    xq_d = nc.dram_tensor("xq_t", [NCH, 128, S], DT, kind="ExternalInput")
    xk_d = nc.dram_tensor("xk_t", [NCH, 128, S], DT, kind="ExternalInput")
    xv_d = nc.dram_tensor("xv_t", [NCH, 128, S], DT, kind="ExternalInput")
    wq_d = nc.dram_tensor("wq", [128, NCH, HPC * DK], DT, kind="ExternalInput")
    wk_d = nc.dram_tensor("wk", [128, NCH, HPC * DK], DT, kind="ExternalInput")
    wv_d = nc.dram_tensor("wv", [128, NCH, HPC * DK], DT, kind="ExternalInput")
    wo_d = nc.dram_tensor("wo", [128, 2, D], DT, kind="ExternalInput")
    tri_d = nc.dram_tensor("tri01", [128, 128], DT, kind="ExternalInput")
    out_d = nc.dram_tensor("out_partial", [S, D], F32, kind="ExternalOutput")
    dbg = {}
    if os.environ.get("KDBG"):
        dbg["qt"] = nc.dram_tensor("qt_dump", [128, 2, S], DT, kind="ExternalOutput")
        dbg["kt"] = nc.dram_tensor("kt_dump", [128, 2, S], DT, kind="ExternalOutput")
        dbg["v"] = nc.dram_tensor("v_dump", [128, NT, HPC * VW], DT, kind="ExternalOutput")
        dbg["ctxt"] = nc.dram_tensor("ctxt_dump", [128, 2, S], DT, kind="ExternalOutput")

    with tile.TileContext(nc) as tc:
        _body(tc, xq_d, xk_d, xv_d, wq_d, wk_d, wv_d, wo_d, tri_d, out_d, dbg)
    nc.compile()
    return nc


def _body(tc, xq_d, xk_d, xv_d, wq_d, wk_d, wv_d, wo_d, tri_d, out_d, dbg=None):
    nc = tc.nc
    EXP = mybir.ActivationFunctionType.Exp
    CPY = mybir.ActivationFunctionType.Copy
    MUL = mybir.AluOpType.mult

    with (
        tc.tile_pool(name="consts", bufs=1) as consts,
        tc.tile_pool(name="persist", bufs=1) as persist,
        tc.tile_pool(name="xbufs", bufs=1) as xbufs,
        tc.tile_pool(name="pt", bufs=14) as pt_pool,
        tc.tile_pool(name="raw", bufs=3) as raw_pool,
        tc.tile_pool(name="small", bufs=2) as small,
        tc.tile_pool(name="osb", bufs=2) as ob_pool,
        tc.tile_pool(name="psum_sc", bufs=2, space="PSUM") as sc_pool,
        tc.tile_pool(name="psum_ctx", bufs=2, space="PSUM") as ctx_pool,
        tc.tile_pool(name="psum_fill", bufs=2, space="PSUM") as fill_pool,
    ):
        # ---- constants / persistent activations ----
        wq_sb = consts.tile([128, NCH, HPC * DK], DT)
        wk_sb = consts.tile([128, NCH, HPC * DK], DT)
        wv_sb = consts.tile([128, NCH, HPC * DK], DT)
        wo_sb = consts.tile([128, 2, D], DT)
        tri_sb = consts.tile([128, 128], DT)

        qt_sb = persist.tile([128, 2, S], DT)         # Q^T, pair-major
        kt_sb = persist.tile([128, 2, S], DT)         # K^T
        v_sb = persist.tile([128, NT, HPC * VW], DT)  # V + ones cols
        ctxt_sb = persist.tile([128, 2, S], DT)       # context^T

        xq_sb = xbufs.tile([128, NCH, S], DT)
        xk_sb = xbufs.tile([128, NCH, S], DT)
        xv_sb = xbufs.tile([128, NCH, S], DT)

        # scalar-engine exp-table warmup (runs during the input DMAs)
        scr = small.tile([1, 16], F32, name="scr", tag="scr")
        scr2 = small.tile([1, 16], DT, name="scr2", tag="scr")
        nc.vector.memset(scr[:], 0.0)
        nc.scalar.activation(scr2[:], scr[:], EXP, scale=1.0)

        for hh in range(HPC):
            nc.vector.memset(v_sb[:, :, hh * VW + DK: hh * VW + DK + 1], 1.0)

        # ---- input DMAs: need-ordered 1MB column-block descriptors ----
        def ld_x(sb, dr, q):
            nc.sync.dma_start(
                sb[:, :, ds(512 * q, 512)],
                dr[:, :, ds(512 * q, 512)].rearrange("c p s -> p c s"))

        nc.sync.dma_start(tri_sb[:], tri_d[:])
        nc.sync.dma_start(wk_sb[:], wk_d[:])
        ld_x(xk_sb, xk_d, 0)
        nc.sync.dma_start(wq_sb[:], wq_d[:])
        ld_x(xq_sb, xq_d, 0)
        nc.sync.dma_start(wv_sb[:], wv_d[:])
        ld_x(xv_sb, xv_d, 0)
        ld_x(xq_sb, xq_d, 1)
        ld_x(xk_sb, xk_d, 1)
        nc.sync.dma_start(wo_sb[:], wo_d[:])
        ld_x(xv_sb, xv_d, 1)
        ld_x(xq_sb, xq_d, 2)
        ld_x(xk_sb, xk_d, 2)
        ld_x(xv_sb, xv_d, 2)
        ld_x(xq_sb, xq_d, 3)
        ld_x(xk_sb, xk_d, 3)
        ld_x(xv_sb, xv_d, 3)

        # ---- fill generators (each yield ~= 2 N=512-class matmuls) ----
        def gen_qkproj(dst, w_sb, x_sb, blk, copy_eng):
            ps = [fill_pool.tile([128, BLK], F32, name=f"qk{p}", tag="fill")
                  for p in range(2)]
            for c in range(NCH):
                for p in range(2):
                    nc.tensor.matmul(
                        ps[p][:],
                        lhsT=w_sb[:, c, ds(128 * p, 128)],
                        rhs=x_sb[:, c, ds(BLK * blk, BLK)],
                        start=(c == 0), stop=(c == NCH - 1))
                if c % 2 == 1 and c < NCH - 1:
                    yield
            for p in range(2):
                dstp = dst[:, p, ds(BLK * blk, BLK)]
                if copy_eng == "scalar":
                    nc.scalar.activation(dstp, ps[p][:], CPY)
                else:
                    nc.vector.tensor_copy(dstp, ps[p][:])
            yield

        def gen_vproj(t):
            ps = fill_pool.tile([128, HPC * DK], F32, name="vps", tag="fill")
            for c in range(NCH):
                nc.tensor.matmul(
                    ps[:],
                    lhsT=xv_sb[:, c, ds(128 * t, 128)],
                    rhs=wv_sb[:, c, :],
                    start=(c == 0), stop=(c == NCH - 1))
                if c in (2, 5):
                    yield
            dst = v_sb[:, t, :].rearrange(
                "p (hh e) -> p hh e", hh=HPC)[:, :, 0:DK]
            nc.vector.tensor_copy(dst, ps[:].rearrange(
                "p (hh e) -> p hh e", hh=HPC))
            yield

        def gen_outproj(t):
            pp = [fill_pool.tile([128, BLK], F32, name=f"pp{nb}", tag="fill")
                  for nb in range(2)]
            for cc in range(2):
                for nb in range(2):
                    nc.tensor.matmul(
                        pp[nb][:],
                        lhsT=ctxt_sb[:, cc, ds(128 * t, 128)],
                        rhs=wo_sb[:, cc, ds(512 * nb, 512)],
                        start=(cc == 0), stop=(cc == 1))
                yield
            ob = ob_pool.tile([128, D], F32, name="ob", tag="ob")
            for nb in range(2):
                nc.vector.tensor_copy(ob[:, ds(512 * nb, 512)], pp[nb][:])
            nc.sync.dma_start(out_d[ds(128 * t, 128), :], ob[:])
            yield

        # ---- attention stream ops ----
        def produce(b, hp, skt):
            u = skt - 4 * b
            qlo = 128 * u if u >= 0 else 0
            w = BLK - qlo
            sc = sc_pool.tile([128, 2, BLK], F32, name="sc", tag="sc")
            for h2 in range(2):
                nc.tensor.matmul(
                    sc[:, h2, qlo:BLK],
                    lhsT=kt_sb[ds(64 * h2, 64), hp, ds(128 * skt, 128)],
                    rhs=qt_sb[ds(64 * h2, 64), hp,
                              ds(BLK * b + qlo, w)],
                    start=True, stop=True)
            pt = pt_pool.tile([128, 2, BLK], DT, name="pt", tag="pt")
            nc.scalar.activation(pt[:, :, qlo:BLK], sc[:, :, qlo:BLK],
                                 EXP, scale=0.125)
            if u >= 0:
                for h2 in range(2):
                    nc.vector.tensor_tensor(
                        pt[:, h2, qlo:qlo + 128],
                        pt[:, h2, qlo:qlo + 128], tri_sb[:], MUL)
            return pt

        def consume(b, hp, skt, pt, ctxps):
            u = skt - 4 * b
            qlo = 128 * u if u >= 0 else 0
            last = 4 * b + 3
            for h2 in range(2):
                h = 2 * hp + h2
                if h not in ctxps:
                    ctxps[h] = ctx_pool.tile(
                        [128, BLK], F32, name=f"ctx{h2}", tag="ctx")
                nc.tensor.matmul(
                    ctxps[h][0:VW, qlo:BLK],
                    lhsT=v_sb[:, skt, ds(h * VW, VW)],
                    rhs=pt[:, h2, qlo:BLK],
                    start=(skt == 0), stop=(skt == last))

        def normalize(b, hp, h2, ctxps):
            h = 2 * hp + h2
            sums = small.tile([1, BLK], F32, name="sums", tag="sums")
            nc.vector.tensor_copy(sums[:], ctxps[h][ds(DK, 1), :])
            raw = raw_pool.tile([DK, BLK], F32, name="raw", tag="raw")
            nc.vector.tensor_copy(raw[:], ctxps[h][0:DK, :])
            r = small.tile([1, BLK], F32, name="r", tag="r")
            nc.vector.reciprocal_approx_fast(out=r[:], in_=sums[:])
            bc = small.tile([64, BLK], F32, name="bc", tag="bc")
            nc.gpsimd.partition_broadcast(bc[:], r[:])
            nc.vector.tensor_tensor(
                ctxt_sb[ds(64 * h2, 64), hp, ds(BLK * b, BLK)],
                raw[:], bc[:], MUL)

        # ---- the interleaved schedule ----
        # K/Q proj block 0 first (scalar-engine copies: ACT is idle here)
        for g in gen_qkproj(kt_sb, wk_sb, xk_sb, 0, "scalar"):
            pass
        for g in gen_qkproj(qt_sb, wq_sb, xq_sb, 0, "scalar"):
            pass

        _vp_gen_ids = {}

        def vp(t):
            g = gen_vproj(t)
            _vp_gen_ids[g] = t
            return g

        fills = [
            gen_qkproj(qt_sb, wq_sb, xq_sb, 1, "vector"),
            gen_qkproj(kt_sb, wk_sb, xk_sb, 1, "vector"),
            vp(0), vp(1), vp(2), vp(3),
            gen_qkproj(qt_sb, wq_sb, xq_sb, 2, "vector"),
            gen_qkproj(kt_sb, wk_sb, xk_sb, 2, "vector"),
            vp(4), vp(5), vp(6), vp(7),
            gen_qkproj(qt_sb, wq_sb, xq_sb, 3, "vector"),
            gen_qkproj(kt_sb, wk_sb, xk_sb, 3, "vector"),
            vp(8), vp(9), vp(10), vp(11),
            vp(12), vp(13), vp(14), vp(15),
        ]
        vp_emitted = [False] * NT   # gen_vproj(t) fully stepped
        state = dict(active=None, pending_ops=[])

        def fill_step():
            while True:
                if state["active"] is None:
                    if state["pending_ops"]:
                        state["active"] = state["pending_ops"].pop(0)
                    elif fills:
                        state["active"] = fills.pop(0)
                    else:
                        return False
                try:
                    next(state["active"])
                    return True
                except StopIteration:
                    g = state["active"]
                    state["active"] = None
                    if g in _vp_gen_ids:
                        vp_emitted[_vp_gen_ids[g]] = True

        pairs = [(0, 0), (0, 1), (1, 0), (1, 1),
                 (2, 0), (2, 1), (3, 0), (3, 1)]
        tasks = [(b, hp, skt) for (b, hp) in pairs for skt in range(4 * b + 4)]

        queue = []          # produced-but-unconsumed (b, hp, skt, pt)
        ctx_maps = {}
        done_blocks = set()

        def try_consume(limit):
            n = 0
            while queue and n < limit:
                b, hp, skt, pt = queue[0]
                if not vp_emitted[skt]:
                    return
                queue.pop(0)
                ctxps = ctx_maps.setdefault((b, hp), {})
                consume(b, hp, skt, pt, ctxps)
                n += 1
                if skt == 4 * b + 3:    # pair complete
                    for h2 in range(2):
                        normalize(b, hp, h2, ctxps)
                    del ctx_maps[(b, hp)]
                    if hp == 1:
                        done_blocks.add(b)
                        for t in range(4 * b, 4 * b + 4):
                            state["pending_ops"].append(gen_outproj(t))

        for k, (b, hp, skt) in enumerate(tasks):
            pt = produce(b, hp, skt)
            queue.append((b, hp, skt, pt))
            if k >= 5:
                fill_step()
            if k >= 2:
                try_consume(2)
            if k >= 5:
                fill_step()

        while queue:
            try_consume(2)
            fill_step()
        while fill_step():
            pass
        if dbg:
            nc.sync.dma_start(dbg["qt"][:], qt_sb[:])
            nc.sync.dma_start(dbg["kt"][:], kt_sb[:])
            nc.sync.dma_start(dbg["v"][:], v_sb[:])
            nc.sync.dma_start(dbg["ctxt"][:], ctxt_sb[:])


def _make_tri():
    i = np.arange(128)[:, None]
    j = np.arange(128)[None, :]
    return (i <= j).astype(NP_DT)


def _prep_core_inputs(inputs, core):
    b = core // 4
    h0 = HPC * (core % 4)
    c0, c1 = h0 * DK, (h0 + HPC) * DK
    f32 = np.float32

    def t_chunks(x):  # [S, D] -> [NCH, 128, S]
        xt = np.ascontiguousarray(np.asarray(x, f32).T)
        return xt.reshape(NCH, 128, S).astype(NP_DT)

    return {
        "xq_t": t_chunks(inputs["input_Q"][b]),
        "xk_t": t_chunks(inputs["input_K"][b]),
        "xv_t": t_chunks(inputs["input_V"][b]),
        "wq": np.ascontiguousarray(np.asarray(inputs["W_Q"], f32)[:, c0:c1].reshape(NCH, 128, HPC * DK).transpose(1, 0, 2)).astype(NP_DT),
        "wk": np.ascontiguousarray(np.asarray(inputs["W_K"], f32)[:, c0:c1].reshape(NCH, 128, HPC * DK).transpose(1, 0, 2)).astype(NP_DT),
        "wv": np.ascontiguousarray(np.asarray(inputs["W_V"], f32)[:, c0:c1].reshape(NCH, 128, HPC * DK).transpose(1, 0, 2)).astype(NP_DT),
        "wo": np.ascontiguousarray(np.asarray(inputs["W_O"], f32)[c0:c1, :].reshape(2, 128, D).transpose(1, 0, 2)).astype(NP_DT),
        "tri01": _make_tri(),
    }


def get_program():
    global _CACHED_NC
    if _CACHED_NC is None:
        _CACHED_NC = _build_program()
    return _CACHED_NC


def kernel(**inputs):
    global LAST_RESULTS
    nc = get_program()
    in_maps = [_prep_core_inputs(inputs, core) for core in range(N_CORES)]
    res = bass_utils.run_bass_kernel_spmd(
        nc, in_maps, core_ids=list(range(N_CORES)),
        trace=TRACE or bool(int(os.environ.get("BASS_TRACE", "0") or 0)))
    LAST_RESULTS = res
    out = np.zeros((B, S, D), np.float32)
    for core in range(N_CORES):
        out[core // 4] += res.results[core]["out_partial"]
    return out
